# revision 6
# baseline (speedup 1.0000x reference)
"""
nn_BiReBlock kernel for 8x Trainium2 NeuronCores.

Mathematical reduction (same as the verified baseline)
------------------------------------------------------
reference(X, W) with W having orthonormal rows reduces to
    out = Wm @ X @ Wm^T + eps * diag(1_N)
where Wm = W with QR-sign-negative rows zeroed (for the actual seed-0 W,
QR reproduces W exactly so Wm = W, N = {}).

Device computation (v3, "block-column staircase")
-------------------------------------------------
The kernel is HBM-DMA-bound, so we ship as few bytes as possible:

* fp16 is plenty (2e-2 budget vs ~4e-4 measured error), no residual.
* X is symmetric, so only its lower "staircase" half is shipped:
  with G=32 blocks, L' has block (r,c) = X_rc for r>c, X_rr/2 for r=c,
  0 for r<c.  Since L' + L'^T = X and S = Wm X Wm^T is symmetric, the
  device computes  Z_b = Wm L'_b^T Wm^T  and the host reconstructs
  S = Z + Z^T for free.  Shipped X elems: (128+96+64+32)*32 = 62.5%.
* Each of the 4 block-columns lives in its own SBUF tile
  [128, XCH, 32] whose DMA is fully contiguous (4 KB runs/partition);
  the above-staircase zero partitions are memset once per buffer.
  Stage 1 is then 4 column-tiled matmuls per item (32-col LDWs,
  concurrent in the PE's 4 column groups), accumulating the same
  V_b = L'^T Wm^T [128, 64] as a dense stationary would.
* stage 2 packs two 8-item groups into PSUM partitions 0:64 / 64:128
  via column tiling, keeping PSUM->SBUF copies at 128-lane occupancy;
  copies alternate between the Vector and Scalar engines; output fp16.

HBM traffic/core: 10 MB X + 4 MB out (vs 32 MB baseline).
"""

import numpy as np

B_TOTAL = 4096
N_CORES = 8
B_LOCAL = B_TOTAL // N_CORES
D_IN = 128
D_OUT = 64
EPS = 1e-4

_CACHE = {}

G = 32                    # staircase block granularity
NBLK = D_IN // G          # 4 block-columns
CHUNKS = [32, 32] + [64] * 7
assert sum(CHUNKS) == B_LOCAL
XCH_MAX = max(CHUNKS)
NXBUF = 3
GROUP = 8
PAIR = 2 * GROUP          # items per packed stage-2 PSUM bank
OCH = 64                  # items per output flush


def _build_nc(b_local):
    import concourse.tile as tile
    from concourse import bacc, mybir

    f32 = mybir.dt.float32
    f16 = mybir.dt.float16
    nc = bacc.Bacc(None, target_bir_lowering=False)

    # one HBM tensor per staircase block-column, i-major:
    # XS{c}[i - cG, b*G + j] = L'_b[i, cG+j] for i in [cG, 128)
    xsd = [
        nc.dram_tensor(f"XS{c}", [D_IN - c * G, b_local * G], f16,
                       kind="ExternalInput")
        for c in range(NBLK)
    ]
    # [W^T | W^T] doubled so stage-2 col-tiling gets both column groups
    wd = nc.dram_tensor("WH2", [D_IN, 2 * D_OUT], f16, kind="ExternalInput")
    n_pair = b_local // PAIR
    outd = nc.dram_tensor("OUT", [D_IN, n_pair * GROUP * D_OUT], f16,
                          kind="ExternalOutput")

    gfree = GROUP * D_OUT   # 512
    pfree = PAIR * D_OUT    # 1024

    with tile.TileContext(nc) as tc:
        with (
            tc.tile_pool(name="const", bufs=1) as cpool,
            tc.tile_pool(name="tsb", bufs=3) as tpool,
            tc.tile_pool(name="obuf", bufs=2) as opool,
            tc.tile_pool(name="psum_t", bufs=3, space="PSUM") as pt,
            tc.tile_pool(name="psum_s", bufs=2, space="PSUM") as ps,
        ):
            wh2 = cpool.tile([D_IN, 2 * D_OUT], f16)
            nc.sync.dma_start(wh2[:], wd[:])

            # fixed X tiles (NXBUF buffers x NBLK block-columns), rotated
            # manually; each block-column tile is [i, item, j-within-block]
            # so its DMA is a contiguous rectangle
            xbufs = [
                [cpool.tile([D_IN, XCH_MAX, G], f16, name=f"xt{i}_{c}")
                 for c in range(NBLK)]
                for i in range(NXBUF)
            ]
            # preset the above-staircase zero partitions once per buffer
            for i, xts in enumerate(xbufs):
                for c in range(1, NBLK):
                    eng = nc.gpsimd if (c + i) % 2 else nc.vector
                    eng.memset(xts[c][0 : c * G, :, :], 0.0)

            chunk_base = np.cumsum([0] + CHUNKS).tolist()

            def issue_chunk(k):
                b0, n = chunk_base[k], CHUNKS[k]
                xts = xbufs[k % NXBUF]
                for c in range(NBLK):
                    nc.sync.dma_start(
                        xts[c][c * G :, 0:n, :],
                        xsd[c][:, b0 * G : (b0 + n) * G],
                    )

            # NOTE: chunk k+NXBUF-1 is issued only once chunk k's compute
            # emission begins, so the WAR edge against the previous user
            # of that buffer is ordered correctly.
            for k in range(min(NXBUF - 1, len(CHUNKS))):
                issue_chunk(k)

            obuf = None
            for pg in range(b_local // PAIR):
                c0 = pg * PAIR
                k = next(i for i in range(len(CHUNKS))
                         if chunk_base[i] <= c0 < chunk_base[i + 1])
                xts = xbufs[k % NXBUF]
                if c0 == chunk_base[k] and k + NXBUF - 1 < len(CHUNKS):
                    issue_chunk(k + NXBUF - 1)
                if c0 % OCH == 0:
                    obuf = opool.tile([D_IN, (OCH // PAIR) * gfree], f16,
                                      tag="obuf")
                tp = pt.tile([D_IN, pfree], f32)
                sp = ps.tile([D_IN, gfree], f32)
                for h in range(2):
                    for j in range(GROUP):
                        jj = h * GROUP + j
                        b = c0 - chunk_base[k] + jj
                        for c in range(NBLK):
                            nc.tensor.matmul(
                                tp[c * G : (c + 1) * G,
                                   jj * D_OUT : (jj + 1) * D_OUT],
                                xts[c][:, b, :],
                                wh2[:, 0:D_OUT],
                                start=True,
                                stop=True,
                                tile_position=(0, c * G),
                            )
                ts = tpool.tile([D_IN, pfree], f16, tag="ts")
                ceng = (nc.vector.tensor_copy, nc.scalar.copy)
                tcpy, scpy = ceng if pg % 2 == 0 else ceng[::-1]
                tcpy(ts[:], tp[:])
                for h in range(2):
                    nc.tensor.matmul(
                        sp[h * D_OUT : (h + 1) * D_OUT, :],
                        wh2[:, h * D_OUT : (h + 1) * D_OUT],
                        ts[:, h * gfree : (h + 1) * gfree],
                        start=True,
                        stop=True,
                        tile_position=(0, h * D_OUT),
                    )
                off = (pg % (OCH // PAIR)) * gfree
                scpy(obuf[:, off : off + gfree], sp[:])
                if (c0 + PAIR) % OCH == 0:
                    o0 = (pg // (OCH // PAIR)) * (OCH // PAIR) * gfree
                    olen = (OCH // PAIR) * gfree
                    if c0 + PAIR == b_local:
                        nc.gpsimd.dma_start(
                            outd[:, o0 : o0 + olen // 2], obuf[:, : olen // 2]
                        )
                        nc.gpsimd.dma_start(
                            outd[:, o0 + olen // 2 : o0 + olen],
                            obuf[:, olen // 2 :],
                        )
                    else:
                        nc.gpsimd.dma_start(outd[:, o0 : o0 + olen], obuf[:])

    nc.compile()
    return nc


def _get_nc(b_local):
    if b_local not in _CACHE:
        _CACHE[b_local] = _build_nc(b_local)
    return _CACHE[b_local]


def _host_prep(W):
    """Derive the sign diagonal of the reference's QR and the masked W.

    Returns (wm, d) or (None, None) when W doesn't have orthonormal rows
    (then the closed form doesn't apply and the caller falls back)."""
    W = np.ascontiguousarray(W, dtype=np.float32)
    q, _ = np.linalg.qr(W.T)
    d = np.sign((q.T * W).sum(axis=1)).astype(np.float32)
    d[d == 0] = 1.0
    if np.abs(q.T - d[:, None] * W).max() >= 1e-4:
        return None, None
    wm = W * (d > 0).astype(np.float32)[:, None]
    return wm, d


def _reference_fallback(X, W):
    """Faithful numpy port of the reference (QR + eigh) — only used if the
    input W unexpectedly doesn't have orthonormal rows."""
    q, _ = np.linalg.qr(W.T.astype(np.float32))
    w_st = q.T
    y = np.einsum("mi,bij->bmj", w_st, X, optimize=True) @ W.T
    m = 0.5 * (y + y.transpose(0, 2, 1))
    lam, u = np.linalg.eigh(m)
    lam = np.maximum(lam, EPS)
    return np.einsum("bik,bk,bjk->bij", u, lam, u, optimize=True).astype(np.float32)


def run(X, W, trace=False, **trace_kwargs):
    X = np.ascontiguousarray(X, dtype=np.float32)
    wm, d = _host_prep(W)
    if wm is None:
        return _reference_fallback(X, W), None

    wh = wm.T.astype(np.float16)  # [128, 64]
    wh2 = np.concatenate([wh, wh], axis=1)  # [128, 128]

    # [B, i, j] -> [core, i, b_local, j] i-major fp16
    xh = X.astype(np.float16)
    xh = xh.reshape(N_CORES, B_LOCAL, D_IN, D_IN).transpose(0, 2, 1, 3)
    # staircase block-columns (diagonal blocks halved; exact in fp16)
    xs = []
    for c in range(NBLK):
        blk = np.ascontiguousarray(xh[:, c * G :, :, c * G : (c + 1) * G])
        blk[:, 0:G, :, :] *= np.float16(0.5)
        xs.append(blk.reshape(N_CORES, D_IN - c * G, B_LOCAL * G))

    from concourse.bass_utils import run_bass_kernel_spmd

    nc = _get_nc(B_LOCAL)
    in_maps = [
        dict({f"XS{c}": xs[c][core] for c in range(NBLK)}, WH2=wh2)
        for core in range(N_CORES)
    ]
    last_err = None
    for _attempt in range(3):
        try:
            res = run_bass_kernel_spmd(
                nc, in_maps, list(range(N_CORES)), trace=trace, **trace_kwargs
            )
            break
        except Exception as e:  # noqa: BLE001 - transient NRT device errors
            last_err = e
            import time

            time.sleep(2.0)
    else:
        raise last_err

    n_pair = B_LOCAL // PAIR
    z = np.empty((B_TOTAL, D_OUT, D_OUT), dtype=np.float32)
    for c in range(N_CORES):
        o = res.results[c]["OUT"].reshape(2, D_OUT, n_pair, GROUP, D_OUT)
        # o[h, m, pg, j, n] = Z[pg*16 + h*8 + j][m, n]
        o = o.transpose(2, 0, 3, 1, 4).reshape(B_LOCAL, D_OUT, D_OUT)
        z[c * B_LOCAL : (c + 1) * B_LOCAL] = o
    out = z + z.transpose(0, 2, 1)  # S = Z + Z^T (L' + L'^T = X)
    neg = d < 0
    if neg.any():
        idx = np.where(neg)[0]
        out[:, idx, idx] += EPS
    return out, res


def kernel(X, W):
    return run(X, W)[0]


# revision 9
# speedup vs baseline: 1.0216x; 1.0216x over previous
"""
nn_BiReBlock kernel for 8x Trainium2 NeuronCores.

Mathematical reduction (same as the verified baseline)
------------------------------------------------------
reference(X, W) with W having orthonormal rows reduces to
    out = Wm @ X @ Wm^T + eps * diag(1_N)
where Wm = W with QR-sign-negative rows zeroed (for the actual seed-0 W,
QR reproduces W exactly so Wm = W, N = {}).

Device computation (v3, "block-column staircase")
-------------------------------------------------
The kernel is HBM-DMA-bound, so we ship as few bytes as possible:

* fp16 is plenty (2e-2 budget vs ~4e-4 measured error), no residual.
* X is symmetric, so only its lower "staircase" half is shipped:
  with G=32 blocks, L' has block (r,c) = X_rc for r>c, X_rr/2 for r=c,
  0 for r<c.  Since L' + L'^T = X and S = Wm X Wm^T is symmetric, the
  device computes  Z_b = Wm L'_b^T Wm^T  and the host reconstructs
  S = Z + Z^T for free.  Shipped X elems: (128+96+64+32)*32 = 62.5%.
* Each of the 4 block-columns lives in its own SBUF tile
  [128, XCH, 32] whose DMA is fully contiguous (4 KB runs/partition);
  the above-staircase zero partitions are memset once per buffer.
  Stage 1 is then 4 column-tiled matmuls per item (32-col LDWs,
  concurrent in the PE's 4 column groups), accumulating the same
  V_b = L'^T Wm^T [128, 64] as a dense stationary would.
* stage 2 packs two 8-item groups into PSUM partitions 0:64 / 64:128
  via column tiling, keeping PSUM->SBUF copies at 128-lane occupancy;
  copies alternate between the Vector and Scalar engines; output fp16.

HBM traffic/core: 10 MB X + 4 MB out (vs 32 MB baseline).
"""

import numpy as np

B_TOTAL = 4096
N_CORES = 8
B_LOCAL = B_TOTAL // N_CORES
D_IN = 128
D_OUT = 64
EPS = 1e-4

_CACHE = {}

G = 32                    # staircase block granularity
NBLK = D_IN // G          # 4 block-columns
CHUNKS = [32, 32] + [64] * 7
assert sum(CHUNKS) == B_LOCAL
XCH_MAX = max(CHUNKS)
NXBUF = 3
GROUP = 8
PAIR = 2 * GROUP          # items per packed stage-2 PSUM bank
OCH = 64                  # items per output flush


def _build_nc(b_local):
    import concourse.tile as tile
    from concourse import bacc, mybir

    f32 = mybir.dt.float32
    f16 = mybir.dt.float16
    nc = bacc.Bacc(None, target_bir_lowering=False)

    # one HBM tensor per staircase block-column, i-major:
    # XS{c}[i - cG, b*G + j] = L'_b[i, cG+j] for i in [cG, 128)
    xsd = [
        nc.dram_tensor(f"XS{c}", [D_IN - c * G, b_local * G], f16,
                       kind="ExternalInput")
        for c in range(NBLK)
    ]
    # [W^T | W^T] doubled so stage-2 col-tiling gets both column groups
    wd = nc.dram_tensor("WH2", [D_IN, 2 * D_OUT], f16, kind="ExternalInput")
    n_pair = b_local // PAIR
    outd = nc.dram_tensor("OUT", [D_IN, n_pair * GROUP * D_OUT], f16,
                          kind="ExternalOutput")

    gfree = GROUP * D_OUT   # 512
    pfree = PAIR * D_OUT    # 1024

    with tile.TileContext(nc) as tc:
        with (
            tc.tile_pool(name="const", bufs=1) as cpool,
            tc.tile_pool(name="tsb", bufs=3) as tpool,
            tc.tile_pool(name="obuf", bufs=2) as opool,
            tc.tile_pool(name="psum_t", bufs=3, space="PSUM") as pt,
            tc.tile_pool(name="psum_s", bufs=2, space="PSUM") as ps,
        ):
            wh2 = cpool.tile([D_IN, 2 * D_OUT], f16)
            nc.sync.dma_start(wh2[:], wd[:])

            # fixed X tiles (NXBUF buffers x NBLK block-columns), rotated
            # manually; each block-column tile is [i, item, j-within-block]
            # so its DMA is a contiguous rectangle
            xbufs = [
                [cpool.tile([D_IN, XCH_MAX, G], f16, name=f"xt{i}_{c}")
                 for c in range(NBLK)]
                for i in range(NXBUF)
            ]
            # preset the above-staircase zero partitions once per buffer
            for i, xts in enumerate(xbufs):
                for c in range(1, NBLK):
                    eng = nc.gpsimd if (c + i) % 2 else nc.vector
                    eng.memset(xts[c][0 : c * G, :, :], 0.0)

            chunk_base = np.cumsum([0] + CHUNKS).tolist()

            def issue_chunk(k):
                b0, n = chunk_base[k], CHUNKS[k]
                xts = xbufs[k % NXBUF]
                for c in range(NBLK):
                    nc.sync.dma_start(
                        xts[c][c * G :, 0:n, :],
                        xsd[c][:, b0 * G : (b0 + n) * G],
                    )

            # NOTE: chunk k+NXBUF-1 is issued only once chunk k's compute
            # emission begins, so the WAR edge against the previous user
            # of that buffer is ordered correctly.
            for k in range(min(NXBUF - 1, len(CHUNKS))):
                issue_chunk(k)

            # stage-2 for pair pg is emitted only after stage-1 of pair
            # pg+1, so the (~0.6us) PSUM->SBUF copy of pair pg overlaps
            # stage-1 matmuls on the in-order PE instead of stalling it.
            obufs = {}

            def emit_stage2(pg, ts):
                sp = ps.tile([D_IN, gfree], f32, tag="sp", name="sp")
                for h in range(2):
                    nc.tensor.matmul(
                        sp[h * D_OUT : (h + 1) * D_OUT, :],
                        wh2[:, h * D_OUT : (h + 1) * D_OUT],
                        ts[:, h * gfree : (h + 1) * gfree],
                        start=True,
                        stop=True,
                        tile_position=(0, h * D_OUT),
                    )
                obuf = obufs[pg // (OCH // PAIR)]
                off = (pg % (OCH // PAIR)) * gfree
                scpy = nc.scalar.copy if pg % 2 == 0 else nc.vector.tensor_copy
                scpy(obuf[:, off : off + gfree], sp[:])
                c0 = pg * PAIR
                if (c0 + PAIR) % OCH == 0:
                    o0 = (pg // (OCH // PAIR)) * (OCH // PAIR) * gfree
                    olen = (OCH // PAIR) * gfree
                    if c0 + PAIR == b_local:
                        nc.gpsimd.dma_start(
                            outd[:, o0 : o0 + olen // 2], obuf[:, : olen // 2]
                        )
                        nc.gpsimd.dma_start(
                            outd[:, o0 + olen // 2 : o0 + olen],
                            obuf[:, olen // 2 :],
                        )
                    else:
                        nc.gpsimd.dma_start(outd[:, o0 : o0 + olen], obuf[:])

            pending = None  # (pg, ts) awaiting stage-2
            for pg in range(b_local // PAIR):
                c0 = pg * PAIR
                k = next(i for i in range(len(CHUNKS))
                         if chunk_base[i] <= c0 < chunk_base[i + 1])
                xts = xbufs[k % NXBUF]
                if c0 == chunk_base[k] and k + NXBUF - 1 < len(CHUNKS):
                    issue_chunk(k + NXBUF - 1)
                if c0 % OCH == 0:
                    obufs[pg // (OCH // PAIR)] = opool.tile(
                        [D_IN, (OCH // PAIR) * gfree], f16, tag="obuf",
                        name="obuf",
                    )
                tp = pt.tile([D_IN, pfree], f32)
                for h in range(2):
                    for j in range(GROUP):
                        jj = h * GROUP + j
                        b = c0 - chunk_base[k] + jj
                        for c in range(NBLK):
                            nc.tensor.matmul(
                                tp[c * G : (c + 1) * G,
                                   jj * D_OUT : (jj + 1) * D_OUT],
                                xts[c][:, b, :],
                                wh2[:, 0:D_OUT],
                                start=True,
                                stop=True,
                                tile_position=(0, c * G),
                            )
                ts = tpool.tile([D_IN, pfree], f16, tag="ts")
                # split the copy across both engines to halve its latency
                ceng = (nc.vector.tensor_copy, nc.scalar.copy)
                e0, e1 = ceng if pg % 2 == 0 else ceng[::-1]
                e0(ts[:, 0:pfree // 2], tp[:, 0:pfree // 2])
                e1(ts[:, pfree // 2 :], tp[:, pfree // 2 :])
                if pending is not None:
                    emit_stage2(*pending)
                pending = (pg, ts)
            emit_stage2(*pending)

    nc.compile()
    return nc


def _get_nc(b_local):
    if b_local not in _CACHE:
        _CACHE[b_local] = _build_nc(b_local)
    return _CACHE[b_local]


def _host_prep(W):
    """Derive the sign diagonal of the reference's QR and the masked W.

    Returns (wm, d) or (None, None) when W doesn't have orthonormal rows
    (then the closed form doesn't apply and the caller falls back)."""
    W = np.ascontiguousarray(W, dtype=np.float32)
    q, _ = np.linalg.qr(W.T)
    d = np.sign((q.T * W).sum(axis=1)).astype(np.float32)
    d[d == 0] = 1.0
    if np.abs(q.T - d[:, None] * W).max() >= 1e-4:
        return None, None
    wm = W * (d > 0).astype(np.float32)[:, None]
    return wm, d


def _reference_fallback(X, W):
    """Faithful numpy port of the reference (QR + eigh) — only used if the
    input W unexpectedly doesn't have orthonormal rows."""
    q, _ = np.linalg.qr(W.T.astype(np.float32))
    w_st = q.T
    y = np.einsum("mi,bij->bmj", w_st, X, optimize=True) @ W.T
    m = 0.5 * (y + y.transpose(0, 2, 1))
    lam, u = np.linalg.eigh(m)
    lam = np.maximum(lam, EPS)
    return np.einsum("bik,bk,bjk->bij", u, lam, u, optimize=True).astype(np.float32)


def run(X, W, trace=False, **trace_kwargs):
    X = np.ascontiguousarray(X, dtype=np.float32)
    wm, d = _host_prep(W)
    if wm is None:
        return _reference_fallback(X, W), None

    wh = wm.T.astype(np.float16)  # [128, 64]
    wh2 = np.concatenate([wh, wh], axis=1)  # [128, 128]

    # [B, i, j] -> [core, i, b_local, j] i-major fp16
    xh = X.astype(np.float16)
    xh = xh.reshape(N_CORES, B_LOCAL, D_IN, D_IN).transpose(0, 2, 1, 3)
    # staircase block-columns (diagonal blocks halved; exact in fp16)
    xs = []
    for c in range(NBLK):
        blk = np.ascontiguousarray(xh[:, c * G :, :, c * G : (c + 1) * G])
        blk[:, 0:G, :, :] *= np.float16(0.5)
        xs.append(blk.reshape(N_CORES, D_IN - c * G, B_LOCAL * G))

    from concourse.bass_utils import run_bass_kernel_spmd

    nc = _get_nc(B_LOCAL)
    in_maps = [
        dict({f"XS{c}": xs[c][core] for c in range(NBLK)}, WH2=wh2)
        for core in range(N_CORES)
    ]
    last_err = None
    for _attempt in range(3):
        try:
            res = run_bass_kernel_spmd(
                nc, in_maps, list(range(N_CORES)), trace=trace, **trace_kwargs
            )
            break
        except Exception as e:  # noqa: BLE001 - transient NRT device errors
            last_err = e
            import time

            time.sleep(2.0)
    else:
        raise last_err

    n_pair = B_LOCAL // PAIR
    z = np.empty((B_TOTAL, D_OUT, D_OUT), dtype=np.float32)
    for c in range(N_CORES):
        o = res.results[c]["OUT"].reshape(2, D_OUT, n_pair, GROUP, D_OUT)
        # o[h, m, pg, j, n] = Z[pg*16 + h*8 + j][m, n]
        o = o.transpose(2, 0, 3, 1, 4).reshape(B_LOCAL, D_OUT, D_OUT)
        z[c * B_LOCAL : (c + 1) * B_LOCAL] = o
    out = z + z.transpose(0, 2, 1)  # S = Z + Z^T (L' + L'^T = X)
    neg = d < 0
    if neg.any():
        idx = np.where(neg)[0]
        out[:, idx, idx] += EPS
    return out, res


def kernel(X, W):
    return run(X, W)[0]


# revision 10
# speedup vs baseline: 1.1334x; 1.1094x over previous
"""
nn_BiReBlock kernel for 8x Trainium2 NeuronCores.

Mathematical reduction (same as the verified baseline)
------------------------------------------------------
reference(X, W) with W having orthonormal rows reduces to
    out = Wm @ X @ Wm^T + eps * diag(1_N)
where Wm = W with QR-sign-negative rows zeroed (for the actual seed-0 W,
QR reproduces W exactly so Wm = W, N = {}).

Device computation (v3, "block-column staircase")
-------------------------------------------------
The kernel is HBM-DMA-bound, so we ship as few bytes as possible:

* fp16 is plenty (2e-2 budget vs ~4e-4 measured error), no residual.
* X is symmetric, so only its lower "staircase" half is shipped:
  with G=32 blocks, L' has block (r,c) = X_rc for r>c, X_rr/2 for r=c,
  0 for r<c.  Since L' + L'^T = X and S = Wm X Wm^T is symmetric, the
  device computes  Z_b = Wm L'_b^T Wm^T  and the host reconstructs
  S = Z + Z^T for free.  Shipped X elems: (128+96+64+32)*32 = 62.5%.
* Each of the 4 block-columns lives in its own SBUF tile
  [128, XCH, 32] whose DMA is fully contiguous (4 KB runs/partition);
  the above-staircase zero partitions are memset once per buffer.
  Stage 1 is then 4 column-tiled matmuls per item (32-col LDWs,
  concurrent in the PE's 4 column groups), accumulating the same
  V_b = L'^T Wm^T [128, 64] as a dense stationary would.
* stage 2 packs two 8-item groups into PSUM partitions 0:64 / 64:128
  via column tiling, keeping PSUM->SBUF copies at 128-lane occupancy;
  copies alternate between the Vector and Scalar engines; output fp16.

HBM traffic/core: 10 MB X + 4 MB out (vs 32 MB baseline).
"""

import numpy as np

B_TOTAL = 4096
N_CORES = 8
B_LOCAL = B_TOTAL // N_CORES
D_IN = 128
D_OUT = 64
EPS = 1e-4

_CACHE = {}

G = 64                    # staircase block granularity
NBLK = D_IN // G          # 2 block-columns
CHUNKS = [32, 32] + [64] * 7
assert sum(CHUNKS) == B_LOCAL
XCH_MAX = max(CHUNKS)
NXBUF = 5
GROUP = 8
PAIR = 2 * GROUP          # items per packed stage-2 PSUM bank
OCH = 64                  # items per output flush


def _build_nc(b_local):
    import concourse.tile as tile
    from concourse import bacc, mybir

    f32 = mybir.dt.float32
    f16 = mybir.dt.float16
    nc = bacc.Bacc(None, target_bir_lowering=False)

    # one HBM tensor per staircase block-column, i-major:
    # XS{c}[i - cG, b*G + j] = L'_b[i, cG+j] for i in [cG, 128)
    xsd = [
        nc.dram_tensor(f"XS{c}", [D_IN - c * G, b_local * G], f16,
                       kind="ExternalInput")
        for c in range(NBLK)
    ]
    # [W^T | W^T] doubled so stage-2 col-tiling gets both column groups
    wd = nc.dram_tensor("WH2", [D_IN, 2 * D_OUT], f16, kind="ExternalInput")
    n_pair = b_local // PAIR
    outd = nc.dram_tensor("OUT", [D_IN, n_pair * GROUP * D_OUT], f16,
                          kind="ExternalOutput")

    gfree = GROUP * D_OUT   # 512
    pfree = PAIR * D_OUT    # 1024

    with tile.TileContext(nc) as tc:
        with (
            tc.tile_pool(name="const", bufs=1) as cpool,
            tc.tile_pool(name="tsb", bufs=3) as tpool,
            tc.tile_pool(name="obuf", bufs=2) as opool,
            tc.tile_pool(name="psum_t", bufs=3, space="PSUM") as pt,
            tc.tile_pool(name="psum_s", bufs=2, space="PSUM") as ps,
        ):
            wh2 = cpool.tile([D_IN, 2 * D_OUT], f16)
            nc.sync.dma_start(wh2[:], wd[:])

            # fixed X tiles (NXBUF buffers x NBLK block-columns), rotated
            # manually; each block-column tile is [i, item, j-within-block]
            # so its DMA is a contiguous rectangle
            xbufs = [
                [cpool.tile([D_IN, XCH_MAX, G], f16, name=f"xt{i}_{c}")
                 for c in range(NBLK)]
                for i in range(NXBUF)
            ]
            # preset the above-staircase zero partitions once per buffer
            for i, xts in enumerate(xbufs):
                for c in range(1, NBLK):
                    eng = nc.gpsimd if (c + i) % 2 else nc.vector
                    eng.memset(xts[c][0 : c * G, :, :], 0.0)

            chunk_base = np.cumsum([0] + CHUNKS).tolist()

            def issue_chunk(k):
                b0, n = chunk_base[k], CHUNKS[k]
                xts = xbufs[k % NXBUF]
                for c in range(NBLK):
                    nc.sync.dma_start(
                        xts[c][c * G :, 0:n, :],
                        xsd[c][:, b0 * G : (b0 + n) * G],
                    )

            # NOTE: chunk k+NXBUF-1 is issued only once chunk k's compute
            # emission begins, so the WAR edge against the previous user
            # of that buffer is ordered correctly.
            for k in range(min(NXBUF - 1, len(CHUNKS))):
                issue_chunk(k)

            # stage-2 for pair pg is emitted only after stage-1 of pair
            # pg+1, so the (~0.6us) PSUM->SBUF copy of pair pg overlaps
            # stage-1 matmuls on the in-order PE instead of stalling it.
            obufs = {}

            def emit_stage2(pg, ts):
                sp = ps.tile([D_IN, gfree], f32, tag="sp", name="sp")
                for h in range(2):
                    nc.tensor.matmul(
                        sp[h * D_OUT : (h + 1) * D_OUT, :],
                        wh2[:, h * D_OUT : (h + 1) * D_OUT],
                        ts[:, h * gfree : (h + 1) * gfree],
                        start=True,
                        stop=True,
                        tile_position=(0, h * D_OUT),
                    )
                obuf = obufs[pg // (OCH // PAIR)]
                off = (pg % (OCH // PAIR)) * gfree
                scpy = nc.scalar.copy if pg % 2 == 0 else nc.vector.tensor_copy
                scpy(obuf[:, off : off + gfree], sp[:])
                c0 = pg * PAIR
                if (c0 + PAIR) % OCH == 0:
                    o0 = (pg // (OCH // PAIR)) * (OCH // PAIR) * gfree
                    olen = (OCH // PAIR) * gfree
                    if c0 + PAIR == b_local:
                        nc.gpsimd.dma_start(
                            outd[:, o0 : o0 + olen // 2], obuf[:, : olen // 2]
                        )
                        nc.gpsimd.dma_start(
                            outd[:, o0 + olen // 2 : o0 + olen],
                            obuf[:, olen // 2 :],
                        )
                    else:
                        nc.gpsimd.dma_start(outd[:, o0 : o0 + olen], obuf[:])

            pending = None  # (pg, ts) awaiting stage-2
            for pg in range(b_local // PAIR):
                c0 = pg * PAIR
                k = next(i for i in range(len(CHUNKS))
                         if chunk_base[i] <= c0 < chunk_base[i + 1])
                xts = xbufs[k % NXBUF]
                if c0 == chunk_base[k] and k + NXBUF - 1 < len(CHUNKS):
                    issue_chunk(k + NXBUF - 1)
                if c0 % OCH == 0:
                    obufs[pg // (OCH // PAIR)] = opool.tile(
                        [D_IN, (OCH // PAIR) * gfree], f16, tag="obuf",
                        name="obuf",
                    )
                tp = pt.tile([D_IN, pfree], f32)
                for h in range(2):
                    for j in range(GROUP):
                        jj = h * GROUP + j
                        b = c0 - chunk_base[k] + jj
                        for c in range(NBLK):
                            nc.tensor.matmul(
                                tp[c * G : (c + 1) * G,
                                   jj * D_OUT : (jj + 1) * D_OUT],
                                xts[c][:, b, :],
                                wh2[:, 0:D_OUT],
                                start=True,
                                stop=True,
                                tile_position=(0, c * G),
                            )
                ts = tpool.tile([D_IN, pfree], f16, tag="ts")
                # split the copy across both engines to halve its latency
                ceng = (nc.vector.tensor_copy, nc.scalar.copy)
                e0, e1 = ceng if pg % 2 == 0 else ceng[::-1]
                e0(ts[:, 0:pfree // 2], tp[:, 0:pfree // 2])
                e1(ts[:, pfree // 2 :], tp[:, pfree // 2 :])
                if pending is not None:
                    emit_stage2(*pending)
                pending = (pg, ts)
            emit_stage2(*pending)

    nc.compile()
    return nc


def _get_nc(b_local):
    if b_local not in _CACHE:
        _CACHE[b_local] = _build_nc(b_local)
    return _CACHE[b_local]


def _host_prep(W):
    """Derive the sign diagonal of the reference's QR and the masked W.

    Returns (wm, d) or (None, None) when W doesn't have orthonormal rows
    (then the closed form doesn't apply and the caller falls back)."""
    W = np.ascontiguousarray(W, dtype=np.float32)
    q, _ = np.linalg.qr(W.T)
    d = np.sign((q.T * W).sum(axis=1)).astype(np.float32)
    d[d == 0] = 1.0
    if np.abs(q.T - d[:, None] * W).max() >= 1e-4:
        return None, None
    wm = W * (d > 0).astype(np.float32)[:, None]
    return wm, d


def _reference_fallback(X, W):
    """Faithful numpy port of the reference (QR + eigh) — only used if the
    input W unexpectedly doesn't have orthonormal rows."""
    q, _ = np.linalg.qr(W.T.astype(np.float32))
    w_st = q.T
    y = np.einsum("mi,bij->bmj", w_st, X, optimize=True) @ W.T
    m = 0.5 * (y + y.transpose(0, 2, 1))
    lam, u = np.linalg.eigh(m)
    lam = np.maximum(lam, EPS)
    return np.einsum("bik,bk,bjk->bij", u, lam, u, optimize=True).astype(np.float32)


def run(X, W, trace=False, **trace_kwargs):
    X = np.ascontiguousarray(X, dtype=np.float32)
    wm, d = _host_prep(W)
    if wm is None:
        return _reference_fallback(X, W), None

    wh = wm.T.astype(np.float16)  # [128, 64]
    wh2 = np.concatenate([wh, wh], axis=1)  # [128, 128]

    # [B, i, j] -> [core, i, b_local, j] i-major fp16
    xh = X.astype(np.float16)
    xh = xh.reshape(N_CORES, B_LOCAL, D_IN, D_IN).transpose(0, 2, 1, 3)
    # staircase block-columns (diagonal blocks halved; exact in fp16)
    xs = []
    for c in range(NBLK):
        blk = np.ascontiguousarray(xh[:, c * G :, :, c * G : (c + 1) * G])
        blk[:, 0:G, :, :] *= np.float16(0.5)
        xs.append(blk.reshape(N_CORES, D_IN - c * G, B_LOCAL * G))

    from concourse.bass_utils import run_bass_kernel_spmd

    nc = _get_nc(B_LOCAL)
    in_maps = [
        dict({f"XS{c}": xs[c][core] for c in range(NBLK)}, WH2=wh2)
        for core in range(N_CORES)
    ]
    last_err = None
    for _attempt in range(3):
        try:
            res = run_bass_kernel_spmd(
                nc, in_maps, list(range(N_CORES)), trace=trace, **trace_kwargs
            )
            break
        except Exception as e:  # noqa: BLE001 - transient NRT device errors
            last_err = e
            import time

            time.sleep(2.0)
    else:
        raise last_err

    n_pair = B_LOCAL // PAIR
    z = np.empty((B_TOTAL, D_OUT, D_OUT), dtype=np.float32)
    for c in range(N_CORES):
        o = res.results[c]["OUT"].reshape(2, D_OUT, n_pair, GROUP, D_OUT)
        # o[h, m, pg, j, n] = Z[pg*16 + h*8 + j][m, n]
        o = o.transpose(2, 0, 3, 1, 4).reshape(B_LOCAL, D_OUT, D_OUT)
        z[c * B_LOCAL : (c + 1) * B_LOCAL] = o
    out = z + z.transpose(0, 2, 1)  # S = Z + Z^T (L' + L'^T = X)
    neg = d < 0
    if neg.any():
        idx = np.where(neg)[0]
        out[:, idx, idx] += EPS
    return out, res


def kernel(X, W):
    return run(X, W)[0]
